# revision 1
# baseline (speedup 1.0000x reference)
"""Ensemble-SRN MoE routing kernel for 8 TRN2 NeuronCores.

Strategy: expert-parallel sharding. The 8 experts are axis-aligned octants of
[-1,1]^3 (GRID=(2,2,2)); core e receives exactly the points routed to expert e
(the all-to-all dispatch happens on the host as part of sharding), runs a dense
single-expert 3->64->64->1 ReLU MLP over its (padded) shard, and the host
inverse-permutes the outputs.

Device mapping per core, per "round" of 4096 points (8 tiles x 512):
  L1 (K=3->M=64):  8 concurrent PE sub-array matmuls at tile_position (32i, 64j)
  L2 (K=64->M=64): 2 waves of 4 concurrent quadrant matmuls
  L3 (K=64->M=1):  8 concurrent matmuls writing y back into the (already
                   evacuated) banks 0-1 of the h2 PSUM tile
  PSUM->SBUF relu+bias evacuations are split between VectorE and ScalarE
  (bank-aligned splits so the two engines never touch the same PSUM bank).
"""

import ml_dtypes
import numpy as np

import concourse.bass as bass
import concourse.tile as tile
from concourse import bacc, mybir
from concourse.bass_utils import run_bass_kernel_spmd

F32 = mybir.dt.float32
BF16 = mybir.dt.bfloat16

N_CORES = 8
GRID = (2, 2, 2)
H = 64
F = 512              # points per tile (one PSUM-bank free dim, fp32)
TILES_PER_ROUND = 8
PTS_PER_ROUND = TILES_PER_ROUND * F  # 4096

# tile t -> (i, j) for L1/L2 input side, (a, b) for L2 output / L3 input side
def _tmap(t):
    i, j = t % 4, t // 4
    a, b = i % 2, j + 2 * (i // 2)
    return i, j, a, b


_PROGRAM_CACHE = {}
LAST_RESULTS = None  # BassKernelResults of the last run (for test harness)
LAST_IN_MAPS = None  # per-core input dicts of the last run (for test harness)
LAST_NC = None       # compiled program of the last run (for test harness)


def _build_program(nr, loop_n=None, stage="full"):
    """Build the SPMD program. loop_n (bench only): repeat the whole body
    loop_n times in a hardware For_i so device time can be measured through
    the noisy axon dispatch path by differencing two loop counts."""
    nc = bacc.Bacc(
        "TRN2",
        target_bir_lowering=False,
        debug=False,
        num_devices=N_CORES,
    )
    xT = nc.dram_tensor("xT", [nr, 4, 6, 512], BF16, kind="ExternalInput")
    w1 = nc.dram_tensor("w1", [128, 128], BF16, kind="ExternalInput")
    w2 = nc.dram_tensor("w2", [128, 128], BF16, kind="ExternalInput")
    w3 = nc.dram_tensor("w3", [128, 1], BF16, kind="ExternalInput")
    b1 = nc.dram_tensor("b1", [128, 1], F32, kind="ExternalInput")
    b2 = nc.dram_tensor("b2", [128, 1], F32, kind="ExternalInput")
    b3 = nc.dram_tensor("b3", [128, 1], F32, kind="ExternalInput")
    yO = nc.dram_tensor("y", [nr, 4, 1024], F32, kind="ExternalOutput")

    RELU = mybir.ActivationFunctionType.Relu
    ADD = mybir.AluOpType.add
    MAX = mybir.AluOpType.max

    with tile.TileContext(nc) as tc:
        with (
            tc.tile_pool(name="const", bufs=1) as const,
            tc.tile_pool(name="xin", bufs=3) as xin,
            tc.tile_pool(name="h1p", bufs=2) as h1pool,
            tc.tile_pool(name="h2p", bufs=2) as h2pool,
            tc.tile_pool(name="yout", bufs=3) as yout,
            tc.tile_pool(name="ps", bufs=4, space="PSUM") as ps,
        ):
            w1_sb = const.tile([128, 128], BF16)
            nc.sync.dma_start(w1_sb[:], w1.ap())
            w2_sb = const.tile([128, 128], BF16)
            nc.sync.dma_start(w2_sb[:], w2.ap())
            w3_sb = const.tile([128, 1], BF16)
            nc.sync.dma_start(w3_sb[:], w3.ap())
            b1_sb = const.tile([128, 1], F32)
            nc.sync.dma_start(b1_sb[:], b1.ap())
            b2_sb = const.tile([128, 1], F32)
            nc.sync.dma_start(b2_sb[:], b2.ap())
            b3_sb = const.tile([128, 1], F32)
            nc.sync.dma_start(b3_sb[:], b3.ap())

            import contextlib
            loop_cm = (
                tc.For_i(
                    0, loop_n, 1,
                    hint_engines=(
                        mybir.EngineType.PE,
                        mybir.EngineType.DVE,
                        mybir.EngineType.Activation,
                        mybir.EngineType.SP,
                    ),
                )
                if loop_n
                else contextlib.nullcontext()
            )
            with loop_cm:
              for r in range(nr):
                  # pair p holds point-tiles t=2p (slot s=0) and t=2p+1 (s=1),
                  # stacked block-diagonally: x rows 3s+c, h rows 64s+j
                  x_sb = xin.tile([128, 512], BF16)
                  for p in range(4):
                      nc.sync.dma_start(
                          x_sb[32 * p : 32 * p + 6, :], xT.ap()[r, p]
                      )

                  # ---- L1: 4 row-tiled block-diag matmuls (concurrent) ----
                  ph1a = ps.tile([128, 1024], F32, tag="hps")  # pairs 0,1
                  ph1b = ps.tile([128, 1024], F32, tag="hps")  # pairs 2,3
                  for p in range(4):
                      dst = ph1a if p < 2 else ph1b
                      nc.tensor.matmul(
                          dst[:, 512 * (p % 2) : 512 * (p % 2) + 512],
                          w1_sb[32 * p : 32 * p + 6, :],
                          x_sb[32 * p : 32 * p + 6, :],
                          start=True,
                          stop=True,
                          tile_position=(32 * p, 0),
                      )
                  # relu + bias evac: DVE takes the a-half, ACT the b-half
                  h1r = h1pool.tile([128, 2048], BF16)
                  nc.vector.tensor_scalar(
                      h1r[:, 0:1024], ph1a[:, 0:1024], b1_sb[:, 0:1], 0.0, ADD, MAX
                  )
                  nc.scalar.activation(
                      h1r[:, 1024:2048], ph1b[:, 0:1024], RELU, bias=b1_sb[:, 0:1]
                  )

                  if stage == "l1":
                      nc.sync.dma_start(
                          yO.ap()[r, 0], h1r[0:1, :].bitcast(F32)
                      )
                      continue

                  # ---- L2: 4 full-array block-diag matmuls ----
                  ph2a = ps.tile([128, 1024], F32, tag="hps")  # pairs 0,1
                  ph2b = ps.tile([128, 1024], F32, tag="hps")  # pairs 2,3
                  for p in range(4):
                      dst = ph2a if p < 2 else ph2b
                      nc.tensor.matmul(
                          dst[:, 512 * (p % 2) : 512 * (p % 2) + 512],
                          w2_sb[:, :],
                          h1r[:, 512 * p : 512 * p + 512],
                          start=True,
                          stop=True,
                          tile_position=(0, 0),
                      )
                  h2r = h2pool.tile([128, 2048], BF16)
                  nc.vector.tensor_scalar(
                      h2r[:, 0:1024], ph2a[:, 0:1024], b2_sb[:, 0:1], 0.0, ADD, MAX
                  )
                  nc.scalar.activation(
                      h2r[:, 1024:2048], ph2b[:, 0:1024], RELU, bias=b2_sb[:, 0:1]
                  )

                  if stage == "l2":
                      nc.sync.dma_start(
                          yO.ap()[r, 0], h2r[0:1, :].bitcast(F32)
                      )
                      continue

                  # ---- L3: 8 tiny matmuls into ph2a (already evacuated) ----
                  for p in range(4):
                      for s in range(2):
                          nc.tensor.matmul(
                              ph2a[32 * p : 32 * p + 1, 512 * s : 512 * s + 512],
                              w3_sb[64 * s : 64 * s + 64, 0:1],
                              h2r[64 * s : 64 * s + 64, 512 * p : 512 * p + 512],
                              start=True,
                              stop=True,
                              tile_position=(64 * s, 32 * p),
                          )
                  y_sb = yout.tile([128, 1024], F32)
                  nc.scalar.activation(
                      y_sb[:], ph2a[:, 0:1024],
                      mybir.ActivationFunctionType.Identity, bias=b3_sb[:, 0:1]
                  )
                  for c in range(4):
                      nc.sync.dma_start(
                          yO.ap()[r, c], y_sb[32 * c : 32 * c + 1, :]
                      )

    nc.compile()
    return nc


def kernel(x, extents_min, extents_max, W1, b1, W2, b2, W3, b3):
    global LAST_RESULTS
    x = np.ascontiguousarray(np.asarray(x, dtype=np.float32))
    extents_min = np.asarray(extents_min, dtype=np.float32)
    extents_max = np.asarray(extents_max, dtype=np.float32)
    W1 = np.asarray(W1, dtype=np.float32)
    b1 = np.asarray(b1, dtype=np.float32)
    W2 = np.asarray(W2, dtype=np.float32)
    b2 = np.asarray(b2, dtype=np.float32)
    W3 = np.asarray(W3, dtype=np.float32)
    b3 = np.asarray(b3, dtype=np.float32)

    n_pts = x.shape[0]
    E = W1.shape[0]
    assert E == N_CORES

    # --- routing (identical fp32 math to the reference) ---
    gvec = np.asarray(GRID, dtype=np.float32)
    u = np.clip((x + np.float32(1.0)) * np.float32(0.5), 0.0, 0.99)
    gi = (u * gvec).astype(np.int32)
    idx = gi[:, 0] + gi[:, 1] * GRID[0] + gi[:, 2] * (GRID[0] * GRID[1])

    order = np.argsort(idx, kind="stable")
    counts = np.bincount(idx, minlength=E)
    starts = np.concatenate([[0], np.cumsum(counts)[:-1]])
    x_sorted = x[order]

    nr = max(1, int(np.ceil(counts.max() / PTS_PER_ROUND)))
    cap = nr * PTS_PER_ROUND

    # --- fold the expert-local normalization into layer-1 weights ---
    # xn = s*x + t, s = 2/(emax-emin), t = -2*emin/(emax-emin) - 1
    span = extents_max - extents_min          # [E, 3]
    s = 2.0 / span
    tvec = -2.0 * extents_min / span - 1.0
    # h1_pre = x @ W1e' + b1e',  W1e' = diag(s) @ W1e, b1e' = b1e + t @ W1e
    W1p = W1 * s[:, :, None]                  # [E, 3, H]
    b1p = b1 + np.einsum("ec,ech->eh", tvec, W1)

    in_maps = []
    for e in range(E):
        xe = np.zeros((cap, 3), dtype=np.float32)
        xe[: counts[e]] = x_sorted[starts[e] : starts[e] + counts[e]]
        # xT[r, p, 3s+c, n] = xe[r*4096 + (2p+s)*512 + n, c]
        xt = (
            xe.reshape(nr, 4, 2, 512, 3)      # r, p, s, n, c
            .transpose(0, 1, 2, 4, 3)         # r, p, s, c, n
            .reshape(nr, 4, 6, 512)
            .astype(ml_dtypes.bfloat16)
        )
        # w1: 4 row strips (one per pair), each the [6,128] block-diag of W1'
        w1e = W1p[e].astype(ml_dtypes.bfloat16)
        w1_full = np.zeros((128, 128), dtype=ml_dtypes.bfloat16)
        for p in range(4):
            w1_full[32 * p : 32 * p + 3, 0:64] = w1e
            w1_full[32 * p + 3 : 32 * p + 6, 64:128] = w1e
        # w2: [128,128] block-diag of W2
        w2_full = np.zeros((128, 128), dtype=ml_dtypes.bfloat16)
        w2_full[0:64, 0:64] = W2[e].astype(ml_dtypes.bfloat16)
        w2_full[64:128, 64:128] = W2[e].astype(ml_dtypes.bfloat16)
        w3_full = np.concatenate([W3[e], W3[e]], axis=0).astype(ml_dtypes.bfloat16)
        b1_full = np.tile(b1p[e], 2)[:, None].astype(np.float32)
        b2_full = np.tile(b2[e], 2)[:, None].astype(np.float32)
        b3_full = np.full((128, 1), b3[e, 0], dtype=np.float32)
        in_maps.append(
            {
                "xT": np.ascontiguousarray(xt),
                "w1": w1_full,
                "w2": w2_full,
                "w3": w3_full,
                "b1": b1_full,
                "b2": b2_full,
                "b3": b3_full,
            }
        )

    if nr not in _PROGRAM_CACHE:
        _PROGRAM_CACHE[nr] = _build_program(nr)
    nc = _PROGRAM_CACHE[nr]

    res = run_bass_kernel_spmd(nc, in_maps, core_ids=list(range(N_CORES)))
    global LAST_IN_MAPS, LAST_NC
    LAST_RESULTS = res
    LAST_IN_MAPS = in_maps
    LAST_NC = nc

    # --- unshard: y_dev[r, p, 512s+n] -> point r*4096 + (2p+s)*512 + n ---
    y_sorted = np.empty(n_pts, dtype=np.float32)
    for e in range(E):
        ye = res.results[e]["y"].reshape(cap)
        y_sorted[starts[e] : starts[e] + counts[e]] = ye[: counts[e]]

    y_full = np.empty(n_pts, dtype=np.float32)
    y_full[order] = y_sorted
    return y_full[:, None]

